# revision 46
# baseline (speedup 1.0000x reference)
"""Trainium2 Bass kernel for batched attention.

Problem: b=16 batches of softmax(Q K^T / sqrt(128)) V with n=m=2048, d=dv=128,
fp32 inputs/outputs.  Sharding: batch dim across 8 NeuronCores (2 per core).

v3.1 design (v1 baseline ~99us, v3 ~96.6us):
  - exp split across TWO engines: ACT does exact exp (fused 1/T scale); DVE
    does a Schraudolph int16 exp approximation in one tensor_scalar
    (round-to-nearest convert verified on HW):
      m = rint(S * 1024*log2(e)/T + (15360 - C_ADJ)); bitcast(m) ~ p.
    Mean bias cancels in softmax; mantissa ripple (~1.8%) enters at
    sqrt(share); per-batch shares tuned with an exact numpy replica of the
    pipeline (validated to 4 digits against HW).
  - batch-0 Q/K transposed on PE (latency-critical), 8 to a PSUM bank, one
    batched [128,1024] copy per bank (ACT/DVE alternating). batch-1
    transposed by the DMA XBAR (dma_start_transpose) fully off-critical.
  - MM2 per batch in A/B halves: A = chunks 0..7 -> PSUM -> parked f16
    partial (keeps PE busy inside the same batch's exp window); B = chunks
    8..15 + PE identity-matmul merge of the parked partial into the same
    accumulation group. Ones column of [V|1] gives the denominator.
  - DVE reciprocal; normalize on DVE (phase B) / alternating DVE+ACT (drain);
    drain double-buffers PSUM via the psS pool (idle during the drain).
"""

import math

import numpy as np

B = 16
N_CORES = 8
B_LOC = B // N_CORES  # 2 batches per core
N = 2048
M = 2048
D = 128
NT = N // 128  # 16
MT = M // 128  # 16
TEMP = 11.313708498984761
INV_TEMP = 1.0 / TEMP

ALPHA = 1024.0 * math.log2(math.e) / TEMP
C_ADJ = 50.0
BETA = 15360.0 - C_ADJ

# exp unit (c, h) -> engine.  batch 0 follows its emission order (h0 of
# c0-7, h1 of c0-7, then (c,h) pairs for c8-15), alternating DVE/ACT from
# DVE -> 16/32 units on DVE.  batch 1 uses the modular table, 10/32 on DVE.
N_UNITS = 2 * MT

_B0_EMIT = ([(c, 0) for c in range(8)] + [(c, 1) for c in range(8)]
            + [u for c in range(8, 16) for u in ((c, 0), (c, 1))])
_B0_DVE = {_B0_EMIT[i] for i in range(N_UNITS) if (i * 14) % N_UNITS < 14}
_B1_DVE = {(i // 2, i % 2) for i in range(N_UNITS) if (i * 9) % N_UNITS < 9}
DVE_UNITS = {0: _B0_DVE, 1: _B1_DVE}

_CACHE = {}


def _build():
    import concourse.bacc as bacc
    import concourse.mybir as mybir
    import concourse.tile as tile
    from concourse.masks import make_identity

    f32 = mybir.dt.float32
    f16 = mybir.dt.float16
    i16 = mybir.dt.int16

    nc = bacc.Bacc("TRN2", target_bir_lowering=False, debug=False,
                   num_devices=N_CORES)
    q_dram = nc.dram_tensor("queries", [B_LOC, N, D], f32, kind="ExternalInput")
    k_dram = nc.dram_tensor("keys", [B_LOC, M, D], f32, kind="ExternalInput")
    v_dram = nc.dram_tensor("values", [B_LOC, M, D], f32, kind="ExternalInput")
    o_dram = nc.dram_tensor("out", [B_LOC, N, D], f32, kind="ExternalOutput")

    with tile.TileContext(nc) as tc:
        with (
            tc.tile_pool(name="const", bufs=1) as const_pool,
            tc.tile_pool(name="nat", bufs=4) as nat_pool,
            tc.tile_pool(name="qT", bufs=2) as qT_pool,
            tc.tile_pool(name="kT", bufs=2) as kT_pool,
            tc.tile_pool(name="vo", bufs=2) as vo_pool,
            tc.tile_pool(name="pT", bufs=26) as pT_pool,
            tc.tile_pool(name="oall", bufs=2) as o_pool,
            tc.tile_pool(name="pa", bufs=20) as pa_pool,
            tc.tile_pool(name="small", bufs=8) as small_pool,
            tc.tile_pool(name="psS", bufs=3, space="PSUM") as psS_pool,
            tc.tile_pool(name="psX", bufs=2, space="PSUM") as psX_pool,
        ):
            # ---- tiles ------------------------------------------------
            q_nats, k_nats, vos, qTs, kTs, o_alls = [], [], [], [], [], []
            for b in range(B_LOC):
                q_nats.append(nat_pool.tile([128, N], f16, name="qnat", tag="nat"))
                k_nats.append(nat_pool.tile([128, M], f16, name="knat", tag="nat"))
                vos.append(vo_pool.tile([128, MT * 129], f16, name="vo", tag="vo"))
                qTs.append(qT_pool.tile([128, N], f16, name="qT", tag="qT"))
                kTs.append(kT_pool.tile([128, M], f16, name="kT", tag="kT"))
                o_alls.append(o_pool.tile([128, NT * 128], f32, name="oall",
                                          tag="oall"))
            ident = const_pool.tile([128, 128], f16)
            pTs = {b: {} for b in range(B_LOC)}
            pas = {}

            # ---- DMA helpers (SWDGE cast loads on Pool) ----------------
            def load_rng(dst, srcd, b, c0, nch):
                cs = slice(c0, c0 + nch)
                nc.gpsimd.dma_start(
                    dst[:].rearrange("p (c d) -> p c d", d=128)[:, cs],
                    srcd[b].rearrange("(c p) d -> p c d", p=128)[:, cs])

            def load_v(b):
                vo = vos[b]
                nc.gpsimd.dma_start(
                    vo[:].rearrange("p (c w) -> p c w", w=129)[:, :, 0:128],
                    v_dram[b].rearrange("(c p) d -> p c d", p=128))
                nc.gpsimd.memset(
                    vo[:].rearrange("p (c w) -> p c w", w=129)[:, :, 128:129],
                    1.0)

            # ---- PE transpose of nch chunks into one PSUM bank, 1 copy ---
            def tr_bank(dst, src, c0, nch, eng):
                pst = psX_pool.tile([128, 64 * nch], f32, name="pst", tag="psX")
                for g in range(nch):
                    c = c0 + g
                    nc.tensor.transpose(
                        pst[:, g * 64:(g + 1) * 64].bitcast(f16),
                        src[:, c * 128:(c + 1) * 128], ident[:])
                cols = slice(c0 * 128, (c0 + nch) * 128)
                if eng == "act":
                    nc.scalar.activation(
                        dst[:, cols], pst[:].bitcast(f16),
                        mybir.ActivationFunctionType.Copy)
                else:
                    nc.vector.tensor_copy(dst[:, cols], pst[:].bitcast(f16))

            def xbar(dst, src, half):
                cols = slice(half * 1024, (half + 1) * 1024)
                nc.sync.dma_start_transpose(
                    dst[:, cols].rearrange("k (c n) -> k c n", n=128),
                    src[:, cols])

            # ---- MM1 unit (one chunk-half) + its exp -------------------
            def _exp(b, c, h, pT, src, cols):
                if (c, h) in DVE_UNITS[b]:
                    nc.vector.tensor_scalar(
                        pT[:, cols].bitcast(i16), src,
                        ALPHA, BETA,
                        mybir.AluOpType.mult, mybir.AluOpType.add)
                else:
                    nc.scalar.activation(
                        pT[:, cols], src,
                        mybir.ActivationFunctionType.Exp, scale=INV_TEMP)

            deferred = []

            def mm1_unit(b, c, h, jsplit=False, defer=False):
                if c not in pTs[b]:
                    pTs[b][c] = pT_pool.tile([128, N], f16, name="pT",
                                             tag="pT")
                pT = pTs[b][c]
                kT, qT = kTs[b], qTs[b]
                psS = psS_pool.tile([128, 1024], f32, name="psS", tag="psS")
                for j in range(2):
                    nc.tensor.matmul(
                        psS[:, j * 512:(j + 1) * 512],
                        kT[:, c * 128:(c + 1) * 128],
                        qT[:, h * 1024 + j * 512:h * 1024 + (j + 1) * 512],
                        start=True, stop=True)
                    if jsplit:
                        _exp(b, c, h, pT, psS[:, j * 512:(j + 1) * 512],
                             slice(h * 1024 + j * 512,
                                   h * 1024 + (j + 1) * 512))
                if defer:
                    deferred.append((b, c, h, pT, psS))
                elif not jsplit:
                    _exp(b, c, h, pT, psS[:],
                         slice(h * 1024, (h + 1) * 1024))

            def mm1_exp(b, c):
                mm1_unit(b, c, 0)
                mm1_unit(b, c, 1)

            # ---- MM2 halves -------------------------------------------
            def mm2_a(b, t):
                """Chunks 0..7 -> PSUM -> parked f16 partial in SBUF."""
                psA = psX_pool.tile([128, 129], f32, name="psA", tag="psX")
                vo = vos[b]
                for c in range(8):
                    nc.tensor.matmul(
                        psA[:],
                        pTs[b][c][:, t * 128:(t + 1) * 128],
                        vo[:, c * 129:(c + 1) * 129],
                        start=(c == 0), stop=(c == 7))
                pa = pa_pool.tile([128, 129], f16, name="pa", tag="pa")
                pas[(b, t)] = pa
                nc.vector.tensor_copy(pa[:], psA[:])

            def mm2_b(b, t, use_psS=False, norm_eng="dve", store_grp=4):
                """Chunks 8..15 + identity-merge of the parked partial,
                reciprocal + normalize + store."""
                pool = psS_pool if use_psS else psX_pool
                psO = pool.tile([128, 129], f32, name="psO",
                                tag="psS" if use_psS else "psX")
                vo = vos[b]
                for c in range(8, MT):
                    nc.tensor.matmul(
                        psO[:],
                        pTs[b][c][:, t * 128:(t + 1) * 128],
                        vo[:, c * 129:(c + 1) * 129],
                        start=(c == 8), stop=False)
                nc.tensor.matmul(
                    psO[:], ident[:], pas[(b, t)][:],
                    start=False, stop=True)
                dst = o_alls[b][:, t * 128:(t + 1) * 128]
                recip = small_pool.tile([128, 1], f32, tag="recip")
                nc.vector.reciprocal(recip[:], psO[:, 128:129])
                if norm_eng == "act":
                    nc.scalar.activation(
                        dst, psO[:, 0:128],
                        mybir.ActivationFunctionType.Copy, scale=recip[:])
                else:
                    nc.vector.tensor_scalar(
                        dst, psO[:, 0:128], recip[:], None,
                        mybir.AluOpType.mult)
                if (t + 1) % store_grp == 0:
                    g0 = t + 1 - store_grp
                    cs = slice(g0, t + 1)
                    nc.sync.dma_start(
                        o_dram[b].rearrange("(c p) d -> p c d", p=128)[:, cs],
                        o_alls[b][:].rearrange("p (c d) -> p c d", d=128)[:, cs])

            # ================= program ==================================
            # batch-0 critical loads first, identity after, rest follow.
            load_rng(k_nats[0], k_dram, 0, 0, 8)
            load_rng(q_nats[0], q_dram, 0, 0, 8)
            make_identity(nc, ident[:])
            load_rng(q_nats[0], q_dram, 0, 8, 8)
            load_rng(k_nats[0], k_dram, 0, 8, 8)
            load_v(0)
            load_rng(k_nats[1], k_dram, 1, 0, 8)
            load_rng(q_nats[1], q_dram, 1, 0, 8)
            load_rng(k_nats[1], k_dram, 1, 8, 8)
            load_rng(q_nats[1], q_dram, 1, 8, 8)
            load_v(1)

            # batch-1 transposes via DMA XBAR, off the critical path.
            for half in range(2):
                xbar(kTs[1], k_nats[1], half)
            for half in range(2):
                xbar(qTs[1], q_nats[1], half)

            # phase A: batch-0 transposes + MM1/exp + own MM2 A-halves.
            # h0 of chunks 0-7 first (needs only the kT/qT quarters); the
            # first two units are j-split so the exp pipeline primes before
            # the second qT quarter lands; later bank copies slotted where
            # their loads arrive.
            tr_bank(kTs[0], k_nats[0], 0, 8, "act")
            tr_bank(qTs[0], q_nats[0], 0, 8, "dve")
            for c in range(2):
                mm1_unit(0, c, 0, jsplit=True)
            for c in range(2, 4):
                mm1_unit(0, c, 0)
            tr_bank(qTs[0], q_nats[0], 8, 8, "act")
            for c in range(4, 8):
                mm1_unit(0, c, 0)
            tr_bank(kTs[0], k_nats[0], 8, 8, "dve")
            # h1 units with A-halves of the h0 tiles (ready since all h0
            # exps are done) interleaved as PE filler against exp pacing.
            for c in range(8):
                mm1_unit(0, c, 1)
                if c >= 3:
                    mm2_a(0, c - 3)
            a_left = list(range(5, 16))
            for c in range(8, MT):
                mm1_exp(0, c)
                for _ in range(2):
                    if a_left:
                        mm2_a(0, a_left.pop(0))

            # phase B: batch-1 MM1/exp + batch-0 B-halves + batch-1 A-halves.
            # c-iterations paired to reduce big<->small matmul transitions.
            for c0 in range(0, MT, 2):
                for c in (c0, c0 + 1):
                    if c == MT - 1:
                        # last chunk's exps (ACT) deferred into phase-C
                        # engine slack (phase-B ACT is oversubscribed).
                        mm1_unit(1, c, 0, defer=True)
                        mm1_unit(1, c, 1, defer=True)
                    elif c == MT - 2:
                        # (c14,h1) is DVE-assigned: defer it past the first
                        # phase-C reciprocals so they aren't FIFO-blocked
                        # behind the DVE exp spill.
                        mm1_unit(1, c, 0)
                        mm1_unit(1, c, 1, defer=True)
                    else:
                        mm1_exp(1, c)
                for c in (c0, c0 + 1):
                    mm2_b(0, c, use_psS=False, norm_eng="dve")
                    if c >= 8:
                        mm2_a(1, 2 * (c - 8))
                        mm2_a(1, 2 * (c - 8) + 1)

            # phase C: ACT-deferred exps first (ACT is idle here); the
            # DVE-deferred unit goes after the first reciprocals; all norms
            # on ACT so the store chain never queues behind the DVE spill.
            for (db, dc, dh, dpT, dpsS) in deferred:
                if (dc, dh) not in DVE_UNITS[db]:
                    for j in range(2):
                        _exp(db, dc, dh, dpT,
                             dpsS[:, j * 512:(j + 1) * 512],
                             slice(dh * 1024 + j * 512,
                                   dh * 1024 + (j + 1) * 512))
            for t in range(NT):
                mm2_b(1, t, use_psS=(t % 3 != 0), norm_eng="act",
                      store_grp=(4 if t < 12 else (2 if t < 14 else 1)))
                if t == 1:
                    for (db, dc, dh, dpT, dpsS) in deferred:
                        if (dc, dh) in DVE_UNITS[db]:
                            for j in range(2):
                                _exp(db, dc, dh, dpT,
                                     dpsS[:, j * 512:(j + 1) * 512],
                                     slice(dh * 1024 + j * 512,
                                           dh * 1024 + (j + 1) * 512))

    nc.compile()
    return nc


def _get_nc():
    if "nc" not in _CACHE:
        _CACHE["nc"] = _build()
    return _CACHE["nc"]


def _ensure_ntff_hook():
    """concourse's trace path imports antenv.axon_hooks, which this image's
    antenv lacks; register an equivalent shim so tracing works."""
    import sys
    try:
        import antenv.axon_hooks  # noqa: F401
        return
    except ImportError:
        pass
    import types
    mod = types.ModuleType("antenv.axon_hooks")
    hook = [None]
    mod.set_axon_ntff_profile_hook = lambda h: hook.__setitem__(0, h)
    mod.get_axon_ntff_profile_hook = lambda: hook[0]
    sys.modules["antenv.axon_hooks"] = mod
    try:
        from trn_agent_boot.trn_boot import _ntff_profile_via_ctypes
        mod.set_axon_ntff_profile_hook(
            _ntff_profile_via_ctypes("/opt/axon/libaxon_pjrt.so"))
    except Exception:
        pass


def run(queries, keys, values, trace=False, tmpdir=None):
    """Run on 8 cores; returns (output, BassKernelResults)."""
    _ensure_ntff_hook()
    from concourse.bass_utils import run_bass_kernel_spmd

    nc = _get_nc()
    queries = np.ascontiguousarray(queries, dtype=np.float32)
    keys = np.ascontiguousarray(keys, dtype=np.float32)
    values = np.ascontiguousarray(values, dtype=np.float32)
    in_maps = []
    for c in range(N_CORES):
        s = slice(c * B_LOC, (c + 1) * B_LOC)
        in_maps.append({
            "queries": queries[s],
            "keys": keys[s],
            "values": values[s],
        })
    res = run_bass_kernel_spmd(nc, in_maps, core_ids=list(range(N_CORES)),
                               trace=trace, tmpdir=tmpdir)
    out = np.concatenate([res.results[c]["out"] for c in range(N_CORES)], axis=0)
    return out, res


def kernel(queries, keys, values):
    out, _ = run(queries, keys, values)
    return out


# revision 47
# speedup vs baseline: 1.0440x; 1.0440x over previous
"""Trainium2 Bass kernel for batched attention.

Problem: b=16 batches of softmax(Q K^T / sqrt(128)) V with n=m=2048, d=dv=128,
fp32 inputs/outputs.  Sharding: batch dim across 8 NeuronCores (2 per core).

v3.1 design (v1 baseline ~99us, v3 ~96.6us):
  - exp split across TWO engines: ACT does exact exp (fused 1/T scale); DVE
    does a Schraudolph int16 exp approximation in one tensor_scalar
    (round-to-nearest convert verified on HW):
      m = rint(S * 1024*log2(e)/T + (15360 - C_ADJ)); bitcast(m) ~ p.
    Mean bias cancels in softmax; mantissa ripple (~1.8%) enters at
    sqrt(share); per-batch shares tuned with an exact numpy replica of the
    pipeline (validated to 4 digits against HW).
  - batch-0 Q/K transposed on PE (latency-critical), 8 to a PSUM bank, one
    batched [128,1024] copy per bank (ACT/DVE alternating). batch-1
    transposed by the DMA XBAR (dma_start_transpose) fully off-critical.
  - MM2 per batch in A/B halves: A = chunks 0..7 -> PSUM -> parked f16
    partial (keeps PE busy inside the same batch's exp window); B = chunks
    8..15 + PE identity-matmul merge of the parked partial into the same
    accumulation group. Ones column of [V|1] gives the denominator.
  - DVE reciprocal; normalize on DVE (phase B) / alternating DVE+ACT (drain);
    drain double-buffers PSUM via the psS pool (idle during the drain).
"""

import math

import numpy as np

B = 16
N_CORES = 8
B_LOC = B // N_CORES  # 2 batches per core
N = 2048
M = 2048
D = 128
NT = N // 128  # 16
MT = M // 128  # 16
TEMP = 11.313708498984761
INV_TEMP = 1.0 / TEMP

ALPHA = 1024.0 * math.log2(math.e) / TEMP
C_ADJ = 50.0
BETA = 15360.0 - C_ADJ

# exp unit (c, h) -> engine.  batch 0 follows its emission order (h0 of
# c0-7, h1 of c0-7, then (c,h) pairs for c8-15), alternating DVE/ACT from
# DVE -> 16/32 units on DVE.  batch 1 uses the modular table, 10/32 on DVE.
N_UNITS = 2 * MT

_B0_EMIT = ([(c, 0) for c in range(8)] + [(c, 1) for c in range(8)]
            + [u for c in range(8, 16) for u in ((c, 0), (c, 1))])
_B0_DVE = {_B0_EMIT[i] for i in range(N_UNITS) if (i * 14) % N_UNITS < 14}
_B1_DVE = {(i // 2, i % 2) for i in range(N_UNITS) if (i * 9) % N_UNITS < 9}
DVE_UNITS = {0: _B0_DVE, 1: _B1_DVE}

_CACHE = {}


def _build():
    import concourse.bacc as bacc
    import concourse.mybir as mybir
    import concourse.tile as tile
    from concourse.masks import make_identity

    f32 = mybir.dt.float32
    f16 = mybir.dt.float16
    i16 = mybir.dt.int16

    nc = bacc.Bacc("TRN2", target_bir_lowering=False, debug=False,
                   num_devices=N_CORES)
    q_dram = nc.dram_tensor("queries", [B_LOC, N, D], f32, kind="ExternalInput")
    k_dram = nc.dram_tensor("keys", [B_LOC, M, D], f32, kind="ExternalInput")
    v_dram = nc.dram_tensor("values", [B_LOC, M, D], f32, kind="ExternalInput")
    o_dram = nc.dram_tensor("out", [B_LOC, N, D], f32, kind="ExternalOutput")

    with tile.TileContext(nc) as tc:
        with (
            tc.tile_pool(name="const", bufs=1) as const_pool,
            tc.tile_pool(name="nat", bufs=4) as nat_pool,
            tc.tile_pool(name="qT", bufs=2) as qT_pool,
            tc.tile_pool(name="kT", bufs=2) as kT_pool,
            tc.tile_pool(name="vo", bufs=2) as vo_pool,
            tc.tile_pool(name="pT", bufs=26) as pT_pool,
            tc.tile_pool(name="oall", bufs=2) as o_pool,
            tc.tile_pool(name="pa", bufs=20) as pa_pool,
            tc.tile_pool(name="small", bufs=8) as small_pool,
            tc.tile_pool(name="psS", bufs=3, space="PSUM") as psS_pool,
            tc.tile_pool(name="psX", bufs=2, space="PSUM") as psX_pool,
        ):
            # ---- tiles ------------------------------------------------
            q_nats, k_nats, vos, qTs, kTs, o_alls = [], [], [], [], [], []
            for b in range(B_LOC):
                q_nats.append(nat_pool.tile([128, N], f16, name="qnat", tag="nat"))
                k_nats.append(nat_pool.tile([128, M], f16, name="knat", tag="nat"))
                vos.append(vo_pool.tile([128, MT * 129], f16, name="vo", tag="vo"))
                qTs.append(qT_pool.tile([128, N], f16, name="qT", tag="qT"))
                kTs.append(kT_pool.tile([128, M], f16, name="kT", tag="kT"))
                o_alls.append(o_pool.tile([128, NT * 128], f32, name="oall",
                                          tag="oall"))
            ident = const_pool.tile([128, 128], f16)
            pTs = {b: {} for b in range(B_LOC)}
            pas = {}

            # ---- DMA helpers (SWDGE cast loads on Pool) ----------------
            def load_rng(dst, srcd, b, c0, nch):
                cs = slice(c0, c0 + nch)
                nc.gpsimd.dma_start(
                    dst[:].rearrange("p (c d) -> p c d", d=128)[:, cs],
                    srcd[b].rearrange("(c p) d -> p c d", p=128)[:, cs])

            def load_v(b):
                vo = vos[b]
                nc.gpsimd.dma_start(
                    vo[:].rearrange("p (c w) -> p c w", w=129)[:, :, 0:128],
                    v_dram[b].rearrange("(c p) d -> p c d", p=128))
                nc.gpsimd.memset(
                    vo[:].rearrange("p (c w) -> p c w", w=129)[:, :, 128:129],
                    1.0)

            # ---- PE transpose of nch chunks into one PSUM bank, 1 copy ---
            def tr_bank(dst, src, c0, nch, eng):
                pst = psX_pool.tile([128, 64 * nch], f32, name="pst", tag="psX")
                for g in range(nch):
                    c = c0 + g
                    nc.tensor.transpose(
                        pst[:, g * 64:(g + 1) * 64].bitcast(f16),
                        src[:, c * 128:(c + 1) * 128], ident[:])
                cols = slice(c0 * 128, (c0 + nch) * 128)
                if eng == "act":
                    nc.scalar.activation(
                        dst[:, cols], pst[:].bitcast(f16),
                        mybir.ActivationFunctionType.Copy)
                else:
                    nc.vector.tensor_copy(dst[:, cols], pst[:].bitcast(f16))

            def xbar(dst, src, half):
                cols = slice(half * 1024, (half + 1) * 1024)
                nc.sync.dma_start_transpose(
                    dst[:, cols].rearrange("k (c n) -> k c n", n=128),
                    src[:, cols])

            # ---- MM1 unit (one chunk-half) + its exp -------------------
            def _exp(b, c, h, pT, src, cols):
                if (c, h) in DVE_UNITS[b]:
                    nc.vector.tensor_scalar(
                        pT[:, cols].bitcast(i16), src,
                        ALPHA, BETA,
                        mybir.AluOpType.mult, mybir.AluOpType.add)
                else:
                    nc.scalar.activation(
                        pT[:, cols], src,
                        mybir.ActivationFunctionType.Exp, scale=INV_TEMP)

            deferred = []

            def mm1_unit(b, c, h, jsplit=False, defer=False):
                if c not in pTs[b]:
                    pTs[b][c] = pT_pool.tile([128, N], f16, name="pT",
                                             tag="pT")
                pT = pTs[b][c]
                kT, qT = kTs[b], qTs[b]
                psS = psS_pool.tile([128, 1024], f32, name="psS", tag="psS")
                for j in range(2):
                    nc.tensor.matmul(
                        psS[:, j * 512:(j + 1) * 512],
                        kT[:, c * 128:(c + 1) * 128],
                        qT[:, h * 1024 + j * 512:h * 1024 + (j + 1) * 512],
                        start=True, stop=True)
                    if jsplit:
                        _exp(b, c, h, pT, psS[:, j * 512:(j + 1) * 512],
                             slice(h * 1024 + j * 512,
                                   h * 1024 + (j + 1) * 512))
                if defer:
                    deferred.append((b, c, h, pT, psS))
                elif not jsplit:
                    _exp(b, c, h, pT, psS[:],
                         slice(h * 1024, (h + 1) * 1024))

            def mm1_exp(b, c):
                mm1_unit(b, c, 0)
                mm1_unit(b, c, 1)

            # ---- MM2 halves -------------------------------------------
            def mm2_a(b, t):
                """Chunks 0..7 -> PSUM -> parked f16 partial in SBUF."""
                psA = psX_pool.tile([128, 129], f32, name="psA", tag="psX")
                vo = vos[b]
                for c in range(8):
                    nc.tensor.matmul(
                        psA[:],
                        pTs[b][c][:, t * 128:(t + 1) * 128],
                        vo[:, c * 129:(c + 1) * 129],
                        start=(c == 0), stop=(c == 7))
                pa = pa_pool.tile([128, 129], f16, name="pa", tag="pa")
                pas[(b, t)] = pa
                nc.vector.tensor_copy(pa[:], psA[:])

            def mm2_b(b, t, use_psS=False, norm_eng="dve", store_grp=4):
                """Chunks 8..15 + identity-merge of the parked partial,
                reciprocal + normalize + store."""
                pool = psS_pool if use_psS else psX_pool
                psO = pool.tile([128, 129], f32, name="psO",
                                tag="psS" if use_psS else "psX")
                vo = vos[b]
                for c in range(8, MT):
                    nc.tensor.matmul(
                        psO[:],
                        pTs[b][c][:, t * 128:(t + 1) * 128],
                        vo[:, c * 129:(c + 1) * 129],
                        start=(c == 8), stop=False)
                nc.tensor.matmul(
                    psO[:], ident[:], pas[(b, t)][:],
                    start=False, stop=True)
                dst = o_alls[b][:, t * 128:(t + 1) * 128]
                recip = small_pool.tile([128, 1], f32, tag="recip")
                nc.vector.reciprocal(recip[:], psO[:, 128:129])
                if norm_eng == "act":
                    nc.scalar.activation(
                        dst, psO[:, 0:128],
                        mybir.ActivationFunctionType.Copy, scale=recip[:])
                else:
                    nc.vector.tensor_scalar(
                        dst, psO[:, 0:128], recip[:], None,
                        mybir.AluOpType.mult)
                if (t + 1) % store_grp == 0:
                    g0 = t + 1 - store_grp
                    cs = slice(g0, t + 1)
                    nc.sync.dma_start(
                        o_dram[b].rearrange("(c p) d -> p c d", p=128)[:, cs],
                        o_alls[b][:].rearrange("p (c d) -> p c d", d=128)[:, cs])

            # ================= program ==================================
            # batch-0 critical loads first, identity after, rest follow.
            load_rng(k_nats[0], k_dram, 0, 0, 8)
            load_rng(q_nats[0], q_dram, 0, 0, 8)
            make_identity(nc, ident[:])
            load_rng(q_nats[0], q_dram, 0, 8, 8)
            load_rng(k_nats[0], k_dram, 0, 8, 8)
            load_v(0)
            load_rng(k_nats[1], k_dram, 1, 0, 8)
            load_rng(q_nats[1], q_dram, 1, 0, 8)
            load_rng(k_nats[1], k_dram, 1, 8, 8)
            load_rng(q_nats[1], q_dram, 1, 8, 8)
            load_v(1)

            # batch-1 transposes via DMA XBAR, off the critical path.
            for half in range(2):
                xbar(kTs[1], k_nats[1], half)
            for half in range(2):
                xbar(qTs[1], q_nats[1], half)

            # phase A: batch-0 transposes + MM1/exp + own MM2 A-halves.
            # h0 of chunks 0-7 first (needs only the kT/qT quarters); the
            # first two units are j-split so the exp pipeline primes before
            # the second qT quarter lands; later bank copies slotted where
            # their loads arrive.
            tr_bank(kTs[0], k_nats[0], 0, 8, "act")
            tr_bank(qTs[0], q_nats[0], 0, 8, "dve")
            for c in range(2):
                mm1_unit(0, c, 0, jsplit=True)
            for c in range(2, 4):
                mm1_unit(0, c, 0)
            tr_bank(qTs[0], q_nats[0], 8, 8, "act")
            for c in range(4, 8):
                mm1_unit(0, c, 0)
            tr_bank(kTs[0], k_nats[0], 8, 8, "dve")
            # h1 units with A-halves of the h0 tiles (ready since all h0
            # exps are done) interleaved as PE filler against exp pacing.
            for c in range(8):
                mm1_unit(0, c, 1)
                if c >= 3:
                    mm2_a(0, c - 3)
            a_left = list(range(5, 16))
            for c in range(8, MT):
                mm1_exp(0, c)
                for _ in range(2):
                    if a_left:
                        mm2_a(0, a_left.pop(0))

            # phase B: batch-1 MM1/exp + batch-0 B-halves + batch-1 A-halves.
            # c-iterations paired to reduce big<->small matmul transitions.
            for c0 in range(0, MT, 2):
                for c in (c0, c0 + 1):
                    if c == MT - 1:
                        # last chunk's exps (ACT) deferred into phase-C
                        # engine slack (phase-B ACT is oversubscribed).
                        mm1_unit(1, c, 0, defer=True)
                        mm1_unit(1, c, 1, defer=True)
                    elif c == MT - 2:
                        # (c14,h1) is DVE-assigned: defer it past the first
                        # phase-C reciprocals so they aren't FIFO-blocked
                        # behind the DVE exp spill.
                        mm1_unit(1, c, 0)
                        mm1_unit(1, c, 1, defer=True)
                    else:
                        mm1_exp(1, c)
                for c in (c0, c0 + 1):
                    mm2_b(0, c, use_psS=False, norm_eng="dve")
                    if c >= 8:
                        mm2_a(1, 2 * (c - 8))
                        mm2_a(1, 2 * (c - 8) + 1)

            # phase C: ACT-deferred exps first (ACT is idle here); the
            # DVE-deferred unit goes after the first reciprocals; all norms
            # on ACT so the store chain never queues behind the DVE spill.
            for (db, dc, dh, dpT, dpsS) in deferred:
                if (dc, dh) not in DVE_UNITS[db]:
                    _exp(db, dc, dh, dpT, dpsS[:],
                         slice(dh * 1024, (dh + 1) * 1024))
            for t in range(NT):
                mm2_b(1, t, use_psS=(t % 3 != 0), norm_eng="act",
                      store_grp=(4 if t < 12 else (2 if t < 14 else 1)))
                if t == 1:
                    for (db, dc, dh, dpT, dpsS) in deferred:
                        if (dc, dh) in DVE_UNITS[db]:
                            _exp(db, dc, dh, dpT, dpsS[:],
                                 slice(dh * 1024, (dh + 1) * 1024))

    nc.compile()
    return nc


def _get_nc():
    if "nc" not in _CACHE:
        _CACHE["nc"] = _build()
    return _CACHE["nc"]


def _ensure_ntff_hook():
    """concourse's trace path imports antenv.axon_hooks, which this image's
    antenv lacks; register an equivalent shim so tracing works."""
    import sys
    try:
        import antenv.axon_hooks  # noqa: F401
        return
    except ImportError:
        pass
    import types
    mod = types.ModuleType("antenv.axon_hooks")
    hook = [None]
    mod.set_axon_ntff_profile_hook = lambda h: hook.__setitem__(0, h)
    mod.get_axon_ntff_profile_hook = lambda: hook[0]
    sys.modules["antenv.axon_hooks"] = mod
    try:
        from trn_agent_boot.trn_boot import _ntff_profile_via_ctypes
        mod.set_axon_ntff_profile_hook(
            _ntff_profile_via_ctypes("/opt/axon/libaxon_pjrt.so"))
    except Exception:
        pass


def run(queries, keys, values, trace=False, tmpdir=None):
    """Run on 8 cores; returns (output, BassKernelResults)."""
    _ensure_ntff_hook()
    from concourse.bass_utils import run_bass_kernel_spmd

    nc = _get_nc()
    queries = np.ascontiguousarray(queries, dtype=np.float32)
    keys = np.ascontiguousarray(keys, dtype=np.float32)
    values = np.ascontiguousarray(values, dtype=np.float32)
    in_maps = []
    for c in range(N_CORES):
        s = slice(c * B_LOC, (c + 1) * B_LOC)
        in_maps.append({
            "queries": queries[s],
            "keys": keys[s],
            "values": values[s],
        })
    res = run_bass_kernel_spmd(nc, in_maps, core_ids=list(range(N_CORES)),
                               trace=trace, tmpdir=tmpdir)
    out = np.concatenate([res.results[c]["out"] for c in range(N_CORES)], axis=0)
    return out, res


def kernel(queries, keys, values):
    out, _ = run(queries, keys, values)
    return out
